# revision 12
# baseline (speedup 1.0000x reference)
"""CTC total-loss kernel for Trainium2 (8 NeuronCores, Bass/Tile).

Strategy (data-parallel over batch, 4 examples per core):

 * The softmax denominator decouples from the CTC alpha recursion in the
   probability domain:  loss_b = -log(rs) + tilt corrections
   + sum_{t<al} lse[t,b], where rs comes from an UNNORMALIZED recursion
   over exp(acts at lattice labels).  Each core runs two pipelines:
     1. stream its 33.5MB acts slab once, computing per-(t,b) sum(exp(acts))
        with one fused ACT Exp+accum instruction per (128,4096) tile;
     2. run the alpha recursion over the per-example lattice emissions.
 * The alpha recursion is computed as a WAVEFRONT over (time-segment,
   lattice-column) cells.  T=512 is split into H=8 segments of 64 steps;
   segment h of example b lives on partition 4h+b (32 partitions).  Cell
   (h, s) = column s over segment h.  Cells on anti-diagonal w = s + h are
   mutually independent, so each wave is ONE scalar_tensor_tensor (the
   skip/merge u-term) plus ONE tensor_tensor_scan across all segments at
   once: 72 waves x 65 elements replaces 65 columns x 512 serial scan
   elements (~4x less serial DVE work than the 2-half s-major form).
 * Compute-engine SBUF access must start at partition 0/32/64/96, so the
   segment-boundary state cannot hop partitions with a shifted copy.
   Instead the hop is a tiny PE matmul against a shift-permutation matrix
   (PE is otherwise idle) writing slot 0 of the u-tile in PSUM; the scan
   consumes the boundary via a "loader" first element whose emission is
   pinned to 1, so its `initial` is the constant 0 and no shifted SBUF
   APs exist anywhere.  Invalid wavefront cells (ramp-up/down) get
   emission 0, so they compute exact zeros and stay contained.
 * Columns are stored WAVE-ALIGNED (column index c = s + h + 2) so every
   per-wave operand is one rectangular AP; the emission table is built in
   the same layout host-side.
 * f32 dynamic range is controlled by a per-(example, segment) exponential
   tilt, estimated host-side with a cheap normalized f64 proxy recursion
   (512 steps over (32,65) arrays); the device state mass then stays near
   1 at every segment boundary, so no mid-kernel renorm / barrier exists.
   Tilts are folded back into the loss in log domain at finalize.

The device program is input-independent (all data dependence flows through
input tensors), so it SPMDs across the 8 cores and compiles once.
"""

import numpy as np

import concourse.bass as bass
import concourse.bacc as bacc
import concourse.tile as tile
from concourse import mybir

F32 = mybir.dt.float32
BF16 = mybir.dt.bfloat16

T, B, V, LMAX = 512, 32, 4096, 32
NCORES = 8
BC = B // NCORES            # 4 examples per core
S = 2 * LMAX + 1            # 65 lattice states
H = 8                       # time segments
SEG = T // H                # 64 steps per segment
NW = S + H - 1              # 72 anti-diagonal waves
EW = SEG + 1                # wave element count (slot 0 = boundary loader)
CW = EW                     # column width in xall
NCOL = S + H + 1            # wave-aligned columns incl. 2 virtual leaders
P = BC * H                  # 32 partitions used by the recursion
NT = (T * BC) // 128        # 16 stream tiles of (128, V)
ECH = 24                    # E-exp chunk size in waves (3 chunks)
EINV = -1.0e4               # "emission = 0" filler for invalid cells

_CACHE = {}


def _build_nc():
    nc = bacc.Bacc(None)
    acts_d = nc.dram_tensor("acts", [T, BC, V], F32, kind="ExternalInput")
    gsub_d = nc.dram_tensor("gsub", [P, NW * EW], BF16, kind="ExternalInput")
    skipk_d = nc.dram_tensor("skipk", [P, NW], F32, kind="ExternalInput")
    biasv_d = nc.dram_tensor("biasv", [P, 1], F32, kind="ExternalInput")
    mshift_d = nc.dram_tensor("mshift", [P, P], F32, kind="ExternalInput")
    xdump_d = nc.dram_tensor("xdump", [P, NCOL * CW], F32,
                             kind="ExternalOutput")
    sums_d = nc.dram_tensor("sums", [128, NT], F32, kind="ExternalOutput")

    acts_rows = acts_d[:].rearrange("t b v -> (t b) v")     # (2048, 4096)
    nch = (NW + ECH - 1) // ECH

    with tile.TileContext(nc) as tc:
        with (
            tc.tile_pool(name="small", bufs=1) as small,
            tc.tile_pool(name="big", bufs=1) as big,
            tc.tile_pool(name="gload", bufs=2) as gload,
            tc.tile_pool(name="stream", bufs=5) as stream,
            tc.tile_pool(name="exsink", bufs=1) as exsink,
            tc.tile_pool(name="upsum", bufs=4, space="PSUM") as upsum,
        ):
            # ---------------- persistent tiles ----------------
            E = big.tile([P, NW * EW], BF16)       # tilted exp(gathered)
            xall = big.tile([P, NCOL * CW], F32)   # wave-aligned columns

            # small loads ride the gpsimd SWDGE queue so the sync HWDGE
            # queue starts streaming the big acts tiles immediately
            skipk_t = small.tile([P, NW], F32)
            nc.gpsimd.dma_start(out=skipk_t[:], in_=skipk_d[:])
            biasv_t = small.tile([P, 1], F32)
            nc.gpsimd.dma_start(out=biasv_t[:], in_=biasv_d[:])
            mshift_t = small.tile([P, P], F32)
            nc.gpsimd.dma_start(out=mshift_t[:], in_=mshift_d[:])
            zbias = small.tile([128, 1], F32)
            nc.vector.memset(zbias[:], 0.0)
            sums = small.tile([128, NT], F32)

            # init: zero everything, then the alpha_{-1} seed at
            # (group 0, column c=1 == s=-1, slot 0).
            nc.vector.memset(xall[:], 0.0)
            nc.vector.memset(xall[0:BC, CW:CW + 1], 1.0)

            # ---------------- emissions in -> E (chunked) ----------------
            def e_chunk(ci):
                w0 = ci * ECH
                w1 = min(NW, w0 + ECH)
                gch = gload.tile([P, ECH * EW], BF16, tag="gch")
                nc.sync.dma_start(out=gch[:, :(w1 - w0) * EW],
                                  in_=gsub_d[:, w0 * EW:w1 * EW])
                nc.scalar.activation(
                    out=E[:, w0 * EW:w1 * EW], in_=gch[:, :(w1 - w0) * EW],
                    func=mybir.ActivationFunctionType.Exp,
                    bias=biasv_t[:], scale=1.0)

            # ---------------- lse stream tile ----------------
            def s_tile(i):
                xt = stream.tile([128, V], F32, tag="xt")
                nc.sync.dma_start(out=xt[:],
                                  in_=acts_rows[i * 128:(i + 1) * 128, :])
                ex = exsink.tile([128, V], F32, tag="ex")
                nc.scalar.activation(
                    out=ex[:], in_=xt[:],
                    func=mybir.ActivationFunctionType.Exp,
                    bias=zbias[:], scale=1.0,
                    accum_out=sums[:, i:i + 1])

            # interleave: two stream tiles lead (their DMAs dominate the
            # kernel span, so they must start first); E chunks slot in
            # between the next stream tiles, still well ahead of the wave
            # chain's consumption.
            s_tile(0)
            s_tile(1)
            for ci in range(nch):
                e_chunk(ci)
                s_tile(2 + ci)
            for i in range(2 + nch, NT):
                s_tile(i)
            nc.sync.dma_start(out=sums_d[:], in_=sums[:])

            # ---------------- wavefront ----------------
            for w in range(NW):
                cb = (w + 2) * CW
                u = upsum.tile([P, EW], F32, tag="u")
                # u[:, 0] = previous group's boundary state, hopped down
                # 4 partitions through the PE shift matrix.
                nc.tensor.matmul(
                    u[:, 0:1], mshift_t[:],
                    xall[:, (w + 1) * CW + SEG:(w + 1) * CW + SEG + 1],
                    start=True, stop=True)
                # u[:, 1:] = k * x[s-2]_t + x[s-1]_t  (columns c-2, c-1)
                nc.vector.scalar_tensor_tensor(
                    out=u[:, 1:EW],
                    in0=xall[:, w * CW:w * CW + SEG],
                    scalar=skipk_t[:, w:w + 1],
                    in1=xall[:, (w + 1) * CW:(w + 1) * CW + SEG],
                    op0=mybir.AluOpType.mult,
                    op1=mybir.AluOpType.add)
                # x_t = (x_{t-1} + u_t) * E_t ; slot 0 is the loader step
                # (E=1) that turns u[:,0] into the carried-in state.
                nc.vector.tensor_tensor_scan(
                    out=xall[:, cb:cb + EW],
                    data0=u[:, 0:EW],
                    data1=E[:, w * EW:(w + 1) * EW],
                    initial=0.0,
                    op0=mybir.AluOpType.add,
                    op1=mybir.AluOpType.mult)

            # ---------------- dump all columns once ----------------
            nc.sync.dma_start(out=xdump_d[:], in_=xall[:])

    nc.compile()
    return nc


def _get_nc():
    if "nc" not in _CACHE:
        _CACHE["nc"] = _build_nc()
    return _CACHE["nc"]


def host_prep(acts, labels, act_lens, label_lens):
    """Build the 8 per-core input maps + finalize aux data."""
    acts = np.ascontiguousarray(np.asarray(acts, dtype=np.float32))
    labels = np.asarray(labels).astype(np.int64)
    al = np.asarray(act_lens).astype(np.int64)
    ll = np.asarray(label_lens).astype(np.int64)
    offsets = np.cumsum(ll) - ll

    # lattice vocab ids EXT[b, s] and skip mask K[b, s]
    EXT = np.zeros((B, S), np.int64)
    K = np.zeros((B, S), np.float32)
    for b in range(B):
        L = int(ll[b])
        labp = np.zeros(LMAX, np.int64)
        labp[:L] = labels[offsets[b]:offsets[b] + L]
        EXT[b, 1::2] = labp
        K[b, 1] = 1.0
        for jj in range(1, L):
            if labp[jj] != labp[jj - 1]:
                K[b, 2 * jj + 1] = 1.0

    # G[t, b, s] = acts[t, b, EXT[b, s]]
    G = np.take_along_axis(acts, np.broadcast_to(EXT[None], (T, B, S)), axis=2)

    # f64 proxy recursion (normalized each step) -> per-segment mass drift.
    # Columns past each example's true lattice end (s > 2L) get emission 0:
    # otherwise mass keeps flowing past the end state and the per-segment
    # normalization leaves the REAL states ~e^-40 below the junk mass,
    # driving their feeders into f32 flush-to-zero on device.
    EG = np.exp(G.astype(np.float64))
    for b in range(B):
        EG[:, b, 2 * int(ll[b]) + 1:] = 0.0
    Kf = K.astype(np.float64)
    A = np.zeros((B, S), np.float64)
    A[:, 0] = EG[0, :, 0]
    A[:, 1] = EG[0, :, 1]
    logm = np.zeros((B, T), np.float64)
    m = A.sum(1)
    A /= m[:, None]
    logm[:, 0] = np.log(m)
    zer1 = np.zeros((B, 1), np.float64)
    zer2 = np.zeros((B, 2), np.float64)
    for t in range(1, T):
        A1 = np.concatenate([zer1, A[:, :-1]], 1)
        A2 = np.concatenate([zer2, A[:, :-2]], 1)
        A = EG[t] * (A + A1 + Kf * A2)
        m = A.sum(1)
        A /= m[:, None]
        logm[:, t] = np.log(m)
    drift = logm.reshape(B, H, SEG).sum(2)          # (B, H)
    tilt = -drift / SEG                              # bias added per step

    mshift = np.zeros((P, P), np.float32)
    for p in range(P - BC):
        mshift[p, p + BC] = 1.0                      # out[p+4] = in[p]

    in_maps = []
    for k in range(NCORES):
        bsl = slice(k * BC, (k + 1) * BC)
        slab = np.ascontiguousarray(acts[:, bsl, :])
        gsub = np.full((P, NW, EW), EINV, np.float32)
        skipk = np.zeros((P, NW), np.float32)
        biasv = np.zeros((P, 1), np.float32)
        for h in range(H):
            for bl in range(BC):
                p = BC * h + bl
                b = k * BC + bl
                Sb = 2 * int(ll[b]) + 1      # true lattice width
                biasv[p, 0] = tilt[b, h]
                # wave w holds column s = w - h: waves h .. h+Sb-1
                gsub[p, h:h + Sb, 0] = -tilt[b, h]   # loader: exp -> 1
                gsub[p, h:h + Sb, 1:] = \
                    G[SEG * h:SEG * (h + 1), b, :Sb].T
                skipk[p, h:h + S] = K[b, :]
        import ml_dtypes
        in_maps.append({"acts": slab,
                        "gsub": gsub.reshape(P, NW * EW)
                                    .astype(ml_dtypes.bfloat16),
                        "skipk": skipk, "biasv": biasv,
                        "mshift": mshift})
    aux = {"tilt": tilt, "al": al, "ll": ll}
    return in_maps, aux


def host_finalize(results, aux):
    """Assemble the scalar loss from per-core outputs."""
    tilt, al, ll = aux["tilt"], aux["al"], aux["ll"]
    total = np.float64(0.0)
    for k in range(NCORES):
        r = results[k]
        sums = np.asarray(r["sums"], np.float64)          # (128, NT)
        xd = np.asarray(r["xdump"], np.float64)           # (P, NCOL*CW)
        lse_rows = np.log(sums.T.reshape(-1)).reshape(T, BC)
        for bl in range(BC):
            b = k * BC + bl
            L = int(ll[b])
            tstar = int(al[b]) - 1
            hs = tstar // SEG
            slot = tstar - SEG * hs + 1
            part = BC * hs + bl
            c1 = 2 * L + hs + 2
            c2 = 2 * L - 1 + hs + 2
            rs = xd[part, c1 * CW + slot] + xd[part, c2 * CW + slot]
            bsum = SEG * tilt[b, :hs].sum() + slot * tilt[b, hs]
            log_unnorm = np.log(rs) - bsum
            loss_b = -log_unnorm + lse_rows[:tstar + 1, bl].sum()
            total += loss_b
    return np.array([total], dtype=np.float32)


def kernel(acts, labels, act_lens, label_lens):
    from concourse.bass_utils import run_bass_kernel_spmd
    in_maps, aux = host_prep(acts, labels, act_lens, label_lens)
    nc = _get_nc()
    res = run_bass_kernel_spmd(nc, in_maps, list(range(NCORES)))
    return host_finalize(res.results, aux)
